# revision 11
# baseline (speedup 1.0000x reference)
"""AI4DEM contact-force stencil kernel for 8 Trainium2 NeuronCores.

Strategy (z-axis spatial decomposition, zero inter-core communication):
- Each core owns 16 of 128 z-slices; host hands it a 20-slice slab
  (2-cell halo each side, zero-padded at global boundaries).
- Layout: SBUF partitions = y, free = (z, x); x padded to 136, y to 132
  so every engine op runs on full 128 partitions at partition-base 0.
- Newton's-third-law symmetric stencil: 62 of 124 offsets computed; each
  interaction I(i,o) is accumulated at i (+) and scattered to i+o (-).
- Mask/out-of-domain semantics: padded/masked cells get a huge unique
  displacement added to their x-coordinate, so any pair touching them has
  r2 >> CS^2 and the (r2 < CS^2) gate zeroes the force exactly.
- dy (cross-partition) neighbor access: per (block, dy) the needed arrays
  are DMA-restaged from y-padded DRAM slabs at row offset dy (DMA has no
  partition-base constraint). dz/dx shifts are free-dim AP offsets.
- Mixed precision: position diffs in fp32 (cancellation), then bf16 for
  the pairwise chain (DVE 2x mode); velocities are host-prescaled by ETA
  and stored bf16 in two x-parities so every bf16 AP stays 4B-aligned.
- Force accumulation on TensorE: identity matmuls accumulate t = G*d into
  fp32 PSUM; scatter uses shifted-identity matrices for the -dy partition
  shift. VectorE runs the chain, ScalarE squares/sqrt (with KNCS folded
  into the Sqrt scale).
"""
import math
import numpy as np
import ml_dtypes

import concourse.bass as bass
import concourse.mybir as mybir
from concourse import bacc
from concourse.tile import TileContext
from concourse.bass_utils import run_bass_kernel_spmd

Alu = mybir.AluOpType
Act = mybir.ActivationFunctionType
F32 = mybir.dt.float32
BF16 = mybir.dt.bfloat16

# --- physics constants (match reference) ---
CS = 0.05
KN = 600000.0
DT = 0.001
GRAV = 9.8
_ALPHA = -math.log(0.5) / math.pi
_GAMMA = _ALPHA / math.sqrt(_ALPHA**2 + 1.0)
MASS = 4.0 / 3.0 * 3.1416 * CS**3 * 2700.0
ETA = 2.0 * _GAMMA * math.sqrt(KN * MASS / 2.0)
KNCS = KN * CS
SCL = KNCS * KNCS          # folded into ACT Sqrt scale: sqrt(SCL*inv2) = KNCS*inv
CS2 = float(np.float32(CS) * np.float32(CS))
BIG = 1.0e4
C1 = DT / MASS

NZ_G, Y, X = 128, 128, 128
XP, YP, ZS, ZOWN = 136, 132, 20, 16
NB, BS = 2, 8
N_CORES = 8

OFFS = [(dz, dy, dx)
        for dz in range(0, 3) for dy in range(-2, 3) for dx in range(-2, 3)
        if (dz > 0) or (dz == 0 and dy > 0) or (dz == 0 and dy == 0 and dx > 0)]
DY_ORDER = [0, 1, 2, -1, -2]
DY_GROUPS = {dy: [o for o in OFFS if o[1] == dy] for dy in DY_ORDER}
N_OFFS = len(OFFS)
POS = ["pxm", "yg", "zg"]
VELE = ["vxe", "vye", "vze"]


def host_prepare(inputs):
    """Full inputs -> list of 8 per-core in_maps."""
    full = {k: np.asarray(v, dtype=np.float32).reshape(NZ_G, Y, X)
            for k, v in inputs.items()}

    def pad(a):
        return np.pad(a, ((2, 2), (2, 2), (4, 4)))

    xg = pad(full["x_grid"]); yg = pad(full["y_grid"]); zg = pad(full["z_grid"])
    vx = pad(full["vx_grid"]); vy = pad(full["vy_grid"]); vz = pad(full["vz_grid"])
    m = pad(full["mask_grid"])
    zi, yi, xi = np.meshgrid(np.arange(NZ_G + 4, dtype=np.float32),
                             np.arange(YP, dtype=np.float32),
                             np.arange(XP, dtype=np.float32), indexing="ij")
    lin = ((zi * YP + yi) * XP + xi + np.float32(1.0)).astype(np.float32)
    pxm = (xg + np.float32(BIG) * (np.float32(1.0) - m) * lin).astype(np.float32)
    eta = np.float32(ETA)
    vxe = (eta * vx).astype(ml_dtypes.bfloat16)
    vye = (eta * vy).astype(ml_dtypes.bfloat16)
    vze = (eta * vz).astype(ml_dtypes.bfloat16)

    idents = np.stack([np.eye(128, dtype=np.float32)] +
                      [-np.eye(128, k=dy, dtype=np.float32) for dy in range(-2, 3)]
                      ).astype(ml_dtypes.bfloat16)

    arrs = {"pxm": pxm, "yg": yg, "zg": zg, "vx": vx, "vy": vy, "vz": vz,
            "mg": m, "vxe": vxe, "vye": vye, "vze": vze}
    in_maps = []
    for c in range(N_CORES):
        sl = slice(c * ZOWN, c * ZOWN + ZS)
        im = {k: np.ascontiguousarray(a[sl].transpose(1, 0, 2))  # [YP, ZS, XP]
              for k, a in arrs.items()}
        im["idents"] = idents
        in_maps.append(im)
    return in_maps


def build_nc():
    nc = bacc.Bacc("TRN2", target_bir_lowering=False, debug=False,
                   num_devices=N_CORES)
    # register the Sign-bias constant (const APs are init-time only)
    _ct = nc.alloc_sbuf_tensor(f"const-float32-negcs2", [128, 1], F32)
    nc.gpsimd.memset(_ct.ap(), -CS2)
    nc.const_aps.aps[(F32, -CS2)] = _ct.ap()
    nc.all_engine_barrier()
    d = {}
    for k in POS + ["vx", "vy", "vz", "mg"]:
        d[k] = nc.dram_tensor(k, [YP, ZS, XP], F32, kind="ExternalInput")
    for k in VELE:
        d[k] = nc.dram_tensor(k, [YP, ZS, XP], BF16, kind="ExternalInput")
    idents_d = nc.dram_tensor("idents", [6, 128, 128], BF16, kind="ExternalInput")
    out_d = nc.dram_tensor("out", [6, Y, ZOWN, X], F32, kind="ExternalOutput")

    XL, XH = 2, 134            # fixed chain x-window (all ops, width 132)

    with TileContext(nc) as tc:
        with tc.tile_pool(name="const", bufs=1) as cpool, \
             tc.tile_pool(name="cent", bufs=1) as gpool, \
             tc.tile_pool(name="shift", bufs=2) as spool, \
             tc.tile_pool(name="vel", bufs=2) as vpool, \
             tc.tile_pool(name="tmp", bufs=1) as tpool, \
             tc.tile_pool(name="tmp2", bufs=2) as tpool2, \
             tc.tile_pool(name="stg", bufs=2) as stpool, \
             tc.tile_pool(name="psum", bufs=1, space="PSUM") as ppool:

            idt = cpool.tile([128, 6, 128], BF16, tag="idents")
            for j in range(6):
                nc.sync.dma_start(out=idt[:, j, :], in_=idents_d[j])

            for b in range(NB):
                # center tiles: rows [2,130), z [8b, 8b+12)
                C = {}
                for k in POS:
                    t = gpool.tile([128, 12, XP], F32, tag=f"c_{k}")
                    nc.sync.dma_start(out=t[:], in_=d[k][2:130, 8 * b:8 * b + 12, :])
                    C[k] = t
                for k in ["vx", "vy", "vz", "mg"]:
                    t = gpool.tile([128, 8, XP], F32, tag=f"c_{k}")
                    nc.sync.dma_start(out=t[:],
                                      in_=d[k][2:130, 8 * b + 2:8 * b + 10, :])
                    C[k] = t
                for k in VELE:
                    t = gpool.tile([128, 12, XP], BF16, tag=f"c_{k}")
                    nc.sync.dma_start(out=t[:], in_=d[k][2:130, 8 * b:8 * b + 12, :])
                    C[k] = t

                FX = ppool.tile([128, BS, X], F32, tag="fx")
                FY = ppool.tile([128, BS, X], F32, tag="fy")
                FZ = ppool.tile([128, BS, X], F32, tag="fz")
                started = set()
                n_done = 0

                for dy in DY_ORDER:
                    # position neighbors: dy=0 reads center tiles (z offset +2)
                    if dy == 0:
                        SP = C
                        zoff = 2
                    else:
                        SP = {}
                        for k in POS:
                            t = spool.tile([128, 10, XP], F32, tag=f"s_{k}")
                            nc.sync.dma_start(
                                out=t[:],
                                in_=d[k][2 + dy:130 + dy, 2 + 8 * b:12 + 8 * b, :])
                            SP[k] = t
                        zoff = 0
                    # velocity neighbors (bf16): staged for every dy, in two
                    # x-parities so bf16 reads stay 4B-aligned
                    SVE, SVO = {}, {}
                    for k in VELE:
                        te = vpool.tile([128, 10, XP], BF16, tag=f"se_{k}")
                        nc.sync.dma_start(
                            out=te[:],
                            in_=d[k][2 + dy:130 + dy, 2 + 8 * b:12 + 8 * b, :])
                        SVE[k] = te
                        to = vpool.tile([128, 10, XP], BF16, tag=f"so_{k}")
                        nc.sync.dma_start(
                            out=to[:, :, 0:XP - 2],
                            in_=d[k][2 + dy:130 + dy, 2 + 8 * b:12 + 8 * b, 1:XP - 1])
                        SVO[k] = to

                    for (dz, _dy, dx) in DY_GROUPS[dy]:
                        nz = BS + dz
                        zc = 2 - dz

                        def cw(t):      # center window
                            return t[:, zc:zc + nz, XL:XH]

                        def nwp(t):     # position-neighbor window
                            return t[:, zoff:zoff + nz, XL + dx:XH + dx]

                        def nwv(k):     # velocity-neighbor window (parity)
                            if dx % 2 == 0:
                                return SVE[k][:, 0:nz, XL + dx:XH + dx]
                            return SVO[k][:, 0:nz, XL + dx - 1:XH + dx - 1]

                        def tmp(tag, dt=BF16):
                            t = tpool.tile([128, 10, XP], dt, tag=tag)
                            return t

                        def tmp2(tag, dt=BF16):
                            t = tpool2.tile([128, 10, XP], dt, tag=tag)
                            return t

                        def win(t):
                            return t[:, 0:nz, XL:XH]

                        dX = tmp2("dX"); dY_ = tmp2("dY"); dZ = tmp2("dZ")
                        nc.vector.scalar_tensor_tensor(
                            win(dX), cw(C["pxm"]), float(-dx * CS), nwp(SP["pxm"]),
                            op0=Alu.add, op1=Alu.subtract)
                        nc.vector.scalar_tensor_tensor(
                            win(dY_), cw(C["yg"]), float(-dy * CS), nwp(SP["yg"]),
                            op0=Alu.add, op1=Alu.subtract)
                        nc.vector.scalar_tensor_tensor(
                            win(dZ), cw(C["zg"]), float(-dz * CS), nwp(SP["zg"]),
                            op0=Alu.add, op1=Alu.subtract)

                        x2 = tmp("x2", F32); y2 = tmp("y2", F32); z2 = tmp("z2", F32)
                        nc.scalar.activation(win(x2), win(dX), Act.Square)
                        nc.scalar.activation(win(y2), win(dY_), Act.Square)
                        nc.scalar.activation(win(z2), win(dZ), Act.Square)
                        nc.vector.tensor_add(win(x2), win(x2), win(y2))
                        nc.vector.scalar_tensor_tensor(
                            win(x2), win(x2), 1e-20, win(z2),
                            op0=Alu.max, op1=Alu.add)
                        # x2 now holds r2 (f32)
                        inv2 = tmp("inv2", F32)
                        nc.vector.reciprocal_approx_fast(win(inv2), win(x2))
                        sgn = tmp("sgn", F32)
                        nc.scalar.activation(win(sgn), win(x2), Act.Sign,
                                             bias=-CS2)
                        m01 = tmp("m01")
                        nc.scalar.activation(win(m01), win(sgn), Act.Copy,
                                             bias=0.5, scale=-0.5)
                        invK = tmp("invK")
                        nc.scalar.activation(win(invK), win(inv2), Act.Sqrt,
                                             scale=SCL)
                        inv2b = tmp("inv2b")
                        nc.scalar.copy(win(inv2b), win(inv2))

                        dv = tmp("dv"); vp1 = tmp("vp1"); vp2 = tmp("vp2")
                        nc.vector.tensor_sub(win(dv), cw(C["vxe"]), nwv("vxe"))
                        nc.vector.tensor_mul(win(vp1), win(dv), win(dX))
                        dv = tmp("dv")
                        nc.vector.tensor_sub(win(dv), cw(C["vye"]), nwv("vye"))
                        nc.vector.tensor_mul(win(vp2), win(dv), win(dY_))
                        nc.vector.tensor_add(win(vp1), win(vp1), win(vp2))
                        dv = tmp("dv")
                        nc.vector.tensor_sub(win(dv), cw(C["vze"]), nwv("vze"))
                        vp2 = tmp("vp2")
                        nc.vector.tensor_mul(win(vp2), win(dv), win(dZ))
                        nc.vector.tensor_add(win(vp1), win(vp1), win(vp2))
                        # tu = vd * inv2
                        nc.vector.tensor_mul(win(vp1), win(vp1), win(inv2b))
                        # tu2 = tu + KN
                        nc.vector.tensor_scalar(win(vp1), win(vp1), KN, None,
                                                op0=Alu.add)
                        # inner = KNCS*inv - tu2
                        nc.vector.tensor_sub(win(invK), win(invK), win(vp1))
                        G = tmp("G")
                        nc.vector.tensor_mul(win(G), win(invK), win(m01))

                        tX = tmp("tX"); tY = tmp("tY"); tZ = tmp("tZ")
                        nc.vector.tensor_mul(win(tX), win(G), win(dX))
                        nc.vector.tensor_mul(win(tY), win(G), win(dY_))
                        nc.vector.tensor_mul(win(tZ), win(G), win(dZ))

                        last = n_done == N_OFFS - 1
                        for (F, t) in ((FX, tX), (FY, tY), (FZ, tZ)):
                            for half in (0, 1):
                                zr = 4 * half
                                key = (id(F), half)
                                st = key not in started
                                started.add(key)
                                # direct: F[y,z',x'] += t[y, z'+dz, x'+4]
                                nc.tensor.matmul(
                                    F[:, zr:zr + 4, :],
                                    idt[:, 0, :],
                                    t[:, dz + zr:dz + zr + 4, 4:132],
                                    start=st, stop=False)
                                # scatter: F[y,z',x'] -= t[y-dy, z', x'+4-dx]
                                nc.tensor.matmul(
                                    F[:, zr:zr + 4, :],
                                    idt[:, 1 + (dy + 2), :],
                                    t[:, zr:zr + 4, 4 - dx:132 - dx],
                                    start=False, stop=last)
                        n_done += 1

                # ---- integration ----
                zo = slice(2, 10)
                zv = slice(0, 8)
                xs = slice(4, 4 + X)
                mw = C["mg"][:, zv, xs]
                for ci, (F, vname, pname) in enumerate(
                        ((FX, "vx", "pxm"), (FY, "vy", "yg"), (FZ, "vz", "zg"))):
                    scr = stpool.tile([128, BS, X], F32, tag="scr")
                    nc.vector.scalar_tensor_tensor(
                        scr[:], F[:], C1, C[vname][:, zv, xs],
                        op0=Alu.mult, op1=Alu.add)
                    stv = stpool.tile([128, BS, X], F32, tag="stg")
                    if ci == 2:
                        nc.vector.scalar_tensor_tensor(
                            stv[:], scr[:], GRAV * DT, mw,
                            op0=Alu.subtract, op1=Alu.mult)
                    else:
                        nc.vector.tensor_mul(stv[:], scr[:], mw)
                    nc.sync.dma_start(out=out_d[3 + ci, :, 8 * b:8 * b + BS, :],
                                      in_=stv[:])
                    scr2 = stpool.tile([128, BS, X], F32, tag="scr")
                    nc.vector.scalar_tensor_tensor(
                        scr2[:], stv[:], DT, C[pname][:, zo, xs],
                        op0=Alu.mult, op1=Alu.add)
                    stp = stpool.tile([128, BS, X], F32, tag="stg")
                    nc.vector.tensor_mul(stp[:], scr2[:], mw)
                    nc.sync.dma_start(out=out_d[ci, :, 8 * b:8 * b + BS, :],
                                      in_=stp[:])
    nc.compile()
    return nc


_NC_CACHE = []


def get_nc():
    if not _NC_CACHE:
        _NC_CACHE.append(build_nc())
    return _NC_CACHE[0]


def assemble(core_outs):
    full = np.concatenate([o.transpose(0, 2, 1, 3) for o in core_outs], axis=1)
    return full.reshape(6, 1, 1, NZ_G, Y, X)


def kernel(**inputs):
    nc = get_nc()
    in_maps = host_prepare(inputs)
    res = run_bass_kernel_spmd(nc, in_maps, core_ids=list(range(N_CORES)))
    return assemble([res.results[c]["out"] for c in range(N_CORES)])


if __name__ == "__main__":
    rng = np.random.default_rng(0)
    shp = (1, 1, NZ_G, Y, X)
    inputs = {
        "x_grid": rng.random(shp, dtype=np.float32),
        "y_grid": rng.random(shp, dtype=np.float32),
        "z_grid": rng.random(shp, dtype=np.float32),
        "vx_grid": rng.standard_normal(shp, dtype=np.float32),
        "vy_grid": rng.standard_normal(shp, dtype=np.float32),
        "vz_grid": rng.standard_normal(shp, dtype=np.float32),
        "mask_grid": np.ones(shp, dtype=np.float32),
    }
    out = kernel(**inputs)
    print("out shape:", out.shape, "finite:", np.isfinite(out).all())
